# revision 1
# baseline (speedup 1.0000x reference)
"""GATv2 message-passing kernel for 8 Trainium2 NeuronCores (Bass/Tile).

Strategy (edge parallelism over receiver-sorted edges):
  * Sort edges by receiver on the host; receivers fall into 128-node blocks.
  * Deal the blocks to the 8 cores balanced by edge count, so every core owns
    complete receiver segments -> the segment softmax/sums are core-local and
    no collective is needed.  Each core returns its blocks' output rows and
    the host reassembles them.
  * On device, per core: phase A computes h = node_features @ W (bias folded
    out algebraically) into an HBM scratch; phase B streams 512-edge chunks:
    indirect-DMA gather of h[senders], one-hot matrices built with is_equal
    compares, and all adds/transposes/segment-sums done as PE matmuls
    accumulating in PSUM.  Mish/Exp run on the scalar (ACT) engine.
  * Softmax is computed without the segment-max shift: logits here are
    bounded (|logit| < ~15), so exp() cannot overflow fp32 and the result is
    mathematically identical.
"""

import ml_dtypes
import numpy as np

import concourse.bass as bass
import concourse.bacc as bacc
import concourse.tile as tile
from concourse import mybir
from concourse.bass_utils import run_bass_kernel_spmd
from concourse.tile_rust import add_dep_helper

F32 = mybir.dt.float32
BF16 = mybir.dt.bfloat16
I32 = mybir.dt.int32
AF = mybir.ActivationFunctionType
OP = mybir.AluOpType

# The act-table-load pass assigns each activation the FIRST table set whose
# function list contains it.  By default Exp->exp_and_others and
# Ln->natural_log, which puts this kernel's exp/ln chain in different sets
# and forces a 1.28us table reload between almost every ACT op.  Emptying
# those two sets (ids preserved) makes Exp and Ln co-resolve to
# natural_log_exp_and_others and Tanh to sigmoid_and_others: 2 reloads per
# receiver block instead of ~6.
import concourse.hw_specs as _hw_specs
import concourse.bacc as _bacc_mod

if not hasattr(_hw_specs, "_gat_orig"):
    _hw_specs._gat_orig = _hw_specs.get_activation_tables

    def _gat_patched(arch):
        t = dict(_hw_specs._gat_orig(arch))
        for k in ("exp_and_others", "natural_log"):
            if k in t:
                t[k] = set()
        return t

    _hw_specs.get_activation_tables = _gat_patched
    _bacc_mod.get_activation_tables = _gat_patched

N_NODES = 50000
N_EDGES = 800000
IN_DIM = 256
EDGE_DIM = 64
EMBED = 128
HEADS = 8
HEAD_DIM = EMBED // HEADS
P = 128
NCORES = 8
CHUNK_G = 4  # groups (of 128 edges) per processing chunk
PAD_RLOC = 200.0  # sentinel receiver-local id for padding edges (> 127)


# ---------------------------------------------------------------- host plan

def _plan(receivers, senders, n_nodes, ncores):
    """Sort edges by receiver, then by sender within each 128-node receiver
    block (so gathers use monotone addresses and fit int16 index windows);
    deal blocks to cores balanced by edge count; pad every (core, position)
    to a common group count; pick per-(position, chunk) gather base offsets
    shared by all cores."""
    order = np.argsort(receivers, kind="stable").astype(np.int64)
    r_s = receivers[order].astype(np.int64)
    nb = -(-n_nodes // P)
    npos = -(-nb // ncores)
    nb_pad = npos * ncores
    n_pad = nb_pad * P
    cnt = np.bincount(r_s // P, minlength=nb_pad).astype(np.int64)
    estart = np.zeros(nb_pad, np.int64)
    estart[1:] = np.cumsum(cnt)[:-1]
    # sender-sort within each receiver block
    for b in range(nb_pad):
        e0, c = int(estart[b]), int(cnt[b])
        if c > 1:
            seg = order[e0:e0 + c]
            order[e0:e0 + c] = seg[np.argsort(senders[seg], kind="stable")]
    r_s = receivers[order].astype(np.int64)
    gcnt = np.maximum(-(-cnt // P), 1)
    deal = np.argsort(-gcnt, kind="stable")
    blocks = deal.reshape(npos, ncores)  # blocks[pos, core] -> block id
    gpos = gcnt[blocks].max(axis=1)      # groups per position (same all cores)
    goff = np.zeros(npos, np.int64)
    goff[1:] = np.cumsum(gpos)[:-1]
    # per-(position, chunk) gather windows, uniform across cores
    bases = []
    for pos in range(npos):
        gp = int(gpos[pos])
        nch = -(-gp // CHUNK_G)
        lo = np.full(nch, np.iinfo(np.int64).max)
        hi = np.zeros(nch, np.int64)
        for core in range(ncores):
            b = int(blocks[pos, core])
            e0, c = int(estart[b]), int(cnt[b])
            sc = senders[order[e0:e0 + c]].astype(np.int64)
            for ch in range(nch):  # real edges only; pads gather row `base`
                part = sc[ch * CHUNK_G * P:(ch + 1) * CHUNK_G * P]
                if part.size:
                    lo[ch] = min(lo[ch], int(part.min()))
                    hi[ch] = max(hi[ch], int(part.max()))
        lo[lo > hi] = 0  # all-padding chunks
        assert (hi - lo).max() < 32768, \
            f"gather window overflow at pos {pos}: {(hi - lo).max()}"
        bases.append([int(x) for x in lo])
    return dict(order=order, r_s=r_s, cnt=cnt, estart=estart, blocks=blocks,
                gpos=gpos, goff=goff, gtot=int(gpos.sum()),
                ecap=int(gpos.sum()) * P, npos=npos, nb_pad=nb_pad,
                bases=bases, n_pad=n_pad)


def _host_inputs(plan, node_features, edge_features, W_kernel, W_bias,
                 We_kernel, We_bias, a, senders):
    """Build the per-core input maps (all numpy, no math beyond transposes)."""
    npos, gtot, ecap = plan["npos"], plan["gtot"], plan["ecap"]
    n_pad = plan["nb_pad"] * P
    n_nodes, in_dim = node_features.shape
    heads, head_dim = a.shape
    embed = heads * head_dim
    edge_dim = edge_features.shape[1]

    nfT = np.zeros((in_dim, n_pad), np.float32)
    nfT[:, :n_nodes] = node_features.T
    We_aug = np.concatenate(
        [We_kernel, (We_bias + 2.0 * W_bias)[None, :]], axis=0
    ).astype(np.float32)
    A_blk = np.zeros((embed, heads), np.float32)
    for h in range(heads):
        A_blk[h * head_dim:(h + 1) * head_dim, h] = a[h]
    Wb_rep = np.tile(W_bias[None, :], (P, 1)).astype(np.float32)
    identity = np.eye(P, dtype=np.float32)
    iota_row = np.tile(np.arange(P, dtype=np.float32)[None, :], (P, 1))
    iota_col = np.arange(P, dtype=np.float32)[:, None].copy()
    ones_row = np.ones((1, P), np.float32)

    efT_all = np.ascontiguousarray(edge_features[plan["order"]].T)  # [64, E]
    s_sorted = senders[plan["order"]].astype(np.int32)
    rloc_all = (plan["r_s"] - (plan["r_s"] // P) * P).astype(np.float32)

    shared = {
        "nfT": nfT, "W": W_kernel.astype(np.float32), "We_aug": We_aug,
        "A_blk": A_blk, "Wb_rep": Wb_rep, "identity": identity,
        "iota_row": iota_row, "iota_col": iota_col, "ones_row": ones_row,
    }
    in_maps = []
    for core in range(NCORES):
        senders16 = np.zeros((P, gtot * 8), np.int16)
        rloc_col = np.full((P, gtot), PAD_RLOC, np.float32)
        rloc_flat = np.full((1, ecap), PAD_RLOC, np.float32)
        efTa = np.zeros((edge_dim + 1, ecap), np.float32)
        efTa[edge_dim, :] = 1.0
        blocknodes = np.zeros((P, npos), np.int32)
        for pos in range(npos):
            b = int(plan["blocks"][pos, core])
            g0 = int(plan["goff"][pos])
            gp = int(plan["gpos"][pos])
            c = int(plan["cnt"][b])
            e0 = int(plan["estart"][b])
            blocknodes[:, pos] = b * P + np.arange(P)
            col0 = g0 * P
            efTa[:edge_dim, col0:col0 + c] = efT_all[:, e0:e0 + c]
            rloc_flat[0, col0:col0 + c] = rloc_all[e0:e0 + c]
            nch = -(-gp // CHUNK_G)
            for ch in range(nch):
                s_ch = min(CHUNK_G * P, gp * P - ch * CHUNK_G * P)
                base = plan["bases"][pos][ch]
                tmp_s = np.full(s_ch, base, np.int64)  # pads -> row `base`
                r0 = ch * CHUNK_G * P
                nreal = min(max(c - r0, 0), s_ch)
                tmp_s[:nreal] = s_sorted[e0 + r0:e0 + r0 + nreal]
                rel = (tmp_s - base).astype(np.int16)
                blk16 = np.tile(rel.reshape(s_ch // 16, 16).T, (8, 1))
                cb = (g0 * P + ch * CHUNK_G * P) // 16
                senders16[:, cb:cb + s_ch // 16] = blk16
            tmp_r = np.full(gp * P, PAD_RLOC, np.float32)
            tmp_r[:c] = rloc_all[e0:e0 + c]
            rloc_col[:, g0:g0 + gp] = tmp_r.reshape(gp, P).T
        m = dict(shared)
        m.update({"senders16": senders16, "rloc_col": rloc_col,
                  "rloc_flat": rloc_flat, "efTa": efTa,
                  "blocknodes": blocknodes})
        in_maps.append(m)
    return in_maps


# ---------------------------------------------------------------- bass build

def _build(plan, n_pad, in_dim, edge_dim, embed, heads, debug=False,
           repeat=1, parts="full"):
    head_dim = embed // heads
    npos, gtot, ecap = plan["npos"], plan["gtot"], plan["ecap"]
    gpos, goff = plan["gpos"], plan["goff"]
    smax = int(gpos.max()) * P
    UW = embed + heads  # U columns: [weighted sum | denom]

    nc = bacc.Bacc("TRN2")
    t_nfT = nc.dram_tensor("nfT", [in_dim, n_pad], F32, kind="ExternalInput")
    t_W = nc.dram_tensor("W", [in_dim, embed], F32, kind="ExternalInput")
    t_We = nc.dram_tensor("We_aug", [edge_dim + 1, embed], F32,
                          kind="ExternalInput")
    t_A = nc.dram_tensor("A_blk", [embed, heads], F32, kind="ExternalInput")
    t_Wb = nc.dram_tensor("Wb_rep", [P, embed], F32, kind="ExternalInput")
    t_id = nc.dram_tensor("identity", [P, P], F32, kind="ExternalInput")
    t_ior = nc.dram_tensor("iota_row", [P, P], F32, kind="ExternalInput")
    t_ioc = nc.dram_tensor("iota_col", [P, 1], F32, kind="ExternalInput")
    t_ones = nc.dram_tensor("ones_row", [1, P], F32, kind="ExternalInput")
    t_s16 = nc.dram_tensor("senders16", [P, gtot * 8], mybir.dt.int16,
                           kind="ExternalInput")
    t_rlc = nc.dram_tensor("rloc_col", [P, gtot], F32, kind="ExternalInput")
    t_rlf = nc.dram_tensor("rloc_flat", [1, ecap], F32, kind="ExternalInput")
    t_efT = nc.dram_tensor("efTa", [edge_dim + 1, ecap], F32,
                           kind="ExternalInput")
    t_bn = nc.dram_tensor("blocknodes", [P, npos], I32, kind="ExternalInput")
    t_out = nc.dram_tensor("out", [npos * P, embed], F32,
                           kind="ExternalOutput")
    t_h = nc.dram_tensor("h_scratch", [n_pad, embed], F32, kind="Internal")
    t_dbg = None
    if debug:
        t_dbg = nc.dram_tensor("dbg", [6, P, CHUNK_G * P], F32,
                               kind="ExternalOutput")

    with tile.TileContext(nc) as tc:
        with tc.tile_pool(name="const", bufs=1) as cp:
            def cload(t, shape):
                s = cp.tile(shape, t.dtype, tag=f"c_{t.name}")
                nc.sync.dma_start(out=s[:], in_=t[:])
                return s

            W0 = cp.tile([P, embed], F32)
            nc.sync.dma_start(out=W0[:], in_=t_W[0:P, :])
            W1 = cp.tile([P, embed], F32)
            nc.sync.dma_start(out=W1[:], in_=t_W[P:2 * P, :])
            We = cload(t_We, [edge_dim + 1, embed])
            Ab = cload(t_A, [embed, heads])
            Wb = cload(t_Wb, [P, embed])
            idn = cload(t_id, [P, P])
            ior = cload(t_ior, [P, P])
            ioc = cload(t_ioc, [P, 1])
            ones = cload(t_ones, [1, P])
            s16 = cload(t_s16, [P, gtot * 8])
            rlocc = cload(t_rlc, [P, gtot])
            bn = cload(t_bn, [P, npos])

            # ---------------- phase A: h = nf @ W (no bias) ----------------
            for _rep in range(repeat):
              with tc.tile_pool(name=f"ha{_rep}", bufs=6) as hap, \
                      tc.tile_pool(name=f"haps{_rep}", bufs=4, space="PSUM") as hpp:
                  HW_ = 4 * P  # nodes per sweep; 256KB DMAs
                  for nt in range(n_pad // HW_):
                      na = hap.tile([P, HW_], F32, tag="nfT0")
                      nc.sync.dma_start(
                          out=na[:],
                          in_=t_nfT[0:P, nt * HW_:(nt + 1) * HW_])
                      nb_t = hap.tile([P, HW_], F32, tag="nfT1")
                      nc.sync.dma_start(
                          out=nb_t[:],
                          in_=t_nfT[P:2 * P, nt * HW_:(nt + 1) * HW_])
                      hstage = hap.tile([P, HW_], F32, tag="hstage")
                      for t in range(HW_ // P):
                          hp = hpp.tile([P, embed], F32, tag="hps")
                          nc.tensor.matmul(hp[:],
                                           lhsT=na[:, t * P:(t + 1) * P],
                                           rhs=W0[:], start=True, stop=False)
                          nc.tensor.matmul(hp[:],
                                           lhsT=nb_t[:, t * P:(t + 1) * P],
                                           rhs=W1[:], start=False, stop=True)
                          nc.scalar.activation(
                              out=hstage[:, t * embed:(t + 1) * embed],
                              in_=hp[:], func=AF.Copy)
                      out_view = bass.AP(
                          t_h[:].tensor, nt * HW_ * embed,
                          [[embed, P], [P * embed, HW_ // P], [1, embed]])
                      nc.sync.dma_start(out=out_view, in_=hstage[:])

              tc.strict_bb_all_engine_barrier()
              if parts == "a":
                  continue
              if debug:
                  nc.sync.dma_start(out=t_dbg[0, :, 0:embed],
                                    in_=t_h[0:P, :])

              # ---------------- phase B: edge processing ---------------------
              with tc.tile_pool(name=f"eb{_rep}", bufs=6) as ep, \
                      tc.tile_pool(name=f"ebsm{_rep}", bufs=4) as esm, \
                      tc.tile_pool(name=f"ebp{_rep}", bufs=2, space="PSUM") as pp, \
                      tc.tile_pool(name=f"ups{_rep}", bufs=2, space="PSUM") as up:
                  prev_exp_ins = None
                  for pos in range(npos):
                      g_here = int(gpos[pos])
                      g0 = int(goff[pos])
                      Hb = ep.tile([P, embed], F32, tag="Hb")
                      nc.gpsimd.indirect_dma_start(
                          out=Hb[:], out_offset=None, in_=t_h[:],
                          in_offset=bass.IndirectOffsetOnAxis(
                              ap=bn[:, pos:pos + 1], axis=0))
                      rrow = esm.tile([1, smax], F32, tag="rrow")
                      nc.sync.dma_start(
                          out=rrow[0:1, :g_here * P],
                          in_=t_rlf[0:1, g0 * P:(g0 + g_here) * P])
                      Ups = up.tile([P, UW], F32, tag="U")
                      lgb = up.tile([P, ((int(gpos.max()) * heads + 127) // 128)
                                     * 128], F32, tag="lgb", bufs=1)
                      nchunks = -(-g_here // CHUNK_G)
                      es_tiles = []
                      sp_tiles = []
                      xc_tiles = []
                      # --- stage 1: pre-activation x and softplus(x) ---------
                      # (ACT stays on the {exp, ln} table set here)
                      for c in range(nchunks):
                          gc = min(CHUNK_G, g_here - c * CHUNK_G)
                          s = gc * P
                          co = c * CHUNK_G * P       # column offset in block
                          ggl = g0 + c * CHUNK_G     # global group index
                          es = ep.tile([P, CHUNK_G * P], F32, tag="es",
                                       bufs=8)
                          es_tiles.append(es)
                          base = plan["bases"][pos][c]
                          rows = min(n_pad - base, 32768)
                          cb = g0 * 8 + c * CHUNK_G * 8
                          nc.gpsimd.dma_gather(
                              out_ap=es[:, :s].rearrange("p (j e) -> p j e",
                                                         e=embed),
                              in_ap=t_h[base:base + rows, :],
                              idxs_ap=s16[:, cb:cb + s // 16],
                              num_idxs=s, num_idxs_reg=s, elem_size=embed)
                          ef = ep.tile([edge_dim + 1, CHUNK_G * P], F32,
                                       tag="ef")
                          nc.sync.dma_start(
                              out=ef[:, :s],
                              in_=t_efT[:, g0 * P + co:g0 * P + co + s])
                          if parts == "ag":
                              continue
                          rep = pp.tile([P, CHUNK_G * P], F32, tag="rep")
                          nc.tensor.matmul(rep[:, :s], lhsT=ones[:],
                                           rhs=rrow[0:1, co:co + s],
                                           start=True, stop=True)
                          GT = ep.tile([P, CHUNK_G * P], F32, tag="GT")
                          nc.vector.tensor_scalar(
                              out=GT[:, :s], in0=rep[:, :s], scalar1=ioc[:],
                              scalar2=None, op0=OP.is_equal)
                          at = pp.tile([P, CHUNK_G * P], F32, tag="attnT", bufs=3)
                          nc.tensor.matmul(at[:, :s], lhsT=We[:], rhs=ef[:, :s],
                                           start=True, stop=False)
                          nc.tensor.matmul(at[:, :s], lhsT=Hb[:],
                                           rhs=GT[:, :s], start=False,
                                           stop=False)
                          for j in range(gc):
                              nc.tensor.matmul(
                                  at[:, j * P:(j + 1) * P],
                                  lhsT=es[:, j * P:(j + 1) * P], rhs=idn[:],
                                  is_transpose=True, start=False,
                                  stop=(j == gc - 1))
                          # mish(x) = x * tanh(ln(1 + exp(x))) — composed from
                          # table-mapped functions (Mish/Softplus have no
                          # compiler mapping).  xc copies x out of PSUM.
                          xc = ep.tile([P, CHUNK_G * P], F32, tag="xc", bufs=8)
                          xc_tiles.append(xc)
                          nc.vector.tensor_copy(out=xc[:, :s], in_=at[:, :s])
                          vv = ep.tile([P, CHUNK_G * P], F32, tag="vv")
                          v_ins = nc.scalar.activation(out=vv[:, :s],
                                                       in_=at[:, :s],
                                                       func=AF.Exp)
                          if c == 0 and prev_exp_ins is not None:
                              # keep ACT's {exp,ln} ops grouped after the
                              # previous block's {tanh,exp} ops so lower_act
                              # emits 2 table loads per block, not ~7.
                              add_dep_helper(v_ins.ins, prev_exp_ins,
                                             sync=False,
                                             reason="act table grouping")
                          sp = ep.tile([P, CHUNK_G * P], F32, tag="sp", bufs=8)
                          sp_tiles.append(sp)
                          sp_ins = nc.scalar.activation(out=sp[:, :s],
                                                        in_=vv[:, :s],
                                                        func=AF.Ln, bias=1.0)
                          if debug and pos == 0 and c == 0:
                              nc.sync.dma_start(out=t_dbg[1, :, :s],
                                                in_=es[:, :s])
                              nc.sync.dma_start(out=t_dbg[2, :, :s],
                                                in_=GT[:, :s])
                              nc.sync.dma_start(out=t_dbg[5, :, :s],
                                                in_=xc[:, :s])
                      if parts == "ag":
                          continue
                      # --- stage 2: tanh, mish, logits, block exp ------------
                      # (ACT switches to the {tanh, exp} table set)
                      for c in range(nchunks):
                          gc = min(CHUNK_G, g_here - c * CHUNK_G)
                          s = gc * P
                          mi = ep.tile([P, CHUNK_G * P], F32, tag="mish")
                          t_ins = nc.scalar.activation(out=mi[:, :s],
                                                       in_=sp_tiles[c][:, :s],
                                                       func=AF.Tanh)
                          if c == 0:
                              add_dep_helper(t_ins.ins, sp_ins.ins, sync=False,
                                             reason="act table grouping")
                          nc.vector.tensor_tensor(out=mi[:, :s],
                                                  in0=xc_tiles[c][:, :s],
                                                  in1=mi[:, :s], op=OP.mult)
                          if debug and pos == 0 and c == 0:
                              nc.sync.dma_start(out=t_dbg[3, :, :s],
                                                in_=mi[:, :s])
                          for j in range(gc):
                              nc.tensor.matmul(
                                  lgb[:, (c * CHUNK_G + j) * heads:
                                      (c * CHUNK_G + j + 1) * heads],
                                  lhsT=mi[:, j * P:(j + 1) * P], rhs=Ab[:],
                                  start=True, stop=True)
                      exb = esm.tile([P, int(gpos.max()) * heads], F32,
                                     tag="exb")
                      exp_ins = nc.scalar.activation(
                          out=exb[:, :g_here * heads],
                          in_=lgb[:, :g_here * heads], func=AF.Exp)
                      prev_exp_ins = exp_ins.ins
                      if debug and pos == 0:
                          nc.sync.dma_start(out=t_dbg[4, :, :g_here * heads],
                                            in_=exb[:, :g_here * heads])
                      # --- stage 3: weighted scatter-accumulate ---
                      for c in range(nchunks):
                          gc = min(CHUNK_G, g_here - c * CHUNK_G)
                          s = gc * P
                          ggl = g0 + c * CHUNK_G
                          es = es_tiles[c]
                          rb = ep.tile([P, CHUNK_G * UW], F32, tag="rhsb")
                          rb3 = rb[:].rearrange("p (j c) -> p j c", j=CHUNK_G)
                          ex_view = rb3[:, :gc, embed:UW]
                          exb_view = exb[:, c * CHUNK_G * heads:
                                         (c * CHUNK_G + gc) * heads].rearrange(
                              "p (j h) -> p j h", j=gc)
                          nc.vector.tensor_copy(out=ex_view, in_=exb_view)
                          m_view = rb3[:, :gc, 0:embed].rearrange(
                              "p j (h w) -> p j h w", w=head_dim)
                          es_view = es[:, :s].rearrange(
                              "p (j h w) -> p j h w", j=gc, w=head_dim)
                          ex_b = exb_view.to_broadcast([P, gc, heads, head_dim])
                          nc.vector.tensor_tensor(out=m_view, in0=es_view,
                                                  in1=ex_b, op=OP.mult)
                          for j in range(gc):
                              Gt = ep.tile([P, P], F32, tag="G")
                              nc.vector.tensor_scalar(
                                  out=Gt[:], in0=ior[:],
                                  scalar1=rlocc[:, ggl + j:ggl + j + 1],
                                  scalar2=None, op0=OP.is_equal)
                              nc.tensor.matmul(
                                  Ups[:], lhsT=Gt[:],
                                  rhs=rb[:, j * UW:(j + 1) * UW],
                                  start=(c == 0 and j == 0),
                                  stop=(c == nchunks - 1 and j == gc - 1))
                      # ---- block epilogue: out = U / max(denom, eps) + Wb ----
                      dn = ep.tile([P, heads], F32, tag="dn")
                      nc.vector.tensor_scalar(out=dn[:],
                                              in0=Ups[:, embed:UW],
                                              scalar1=1e-30, scalar2=None,
                                              op0=OP.max)
                      rc = ep.tile([P, heads], F32, tag="rc")
                      nc.vector.reciprocal(rc[:], dn[:])
                      nd = ep.tile([P, embed], F32, tag="nodes")
                      ndv = nd[:].rearrange("p (h w) -> p h w", w=head_dim)
                      uv = Ups[:, 0:embed].rearrange("p (h w) -> p h w",
                                                     w=head_dim)
                      rcb = rc[:].to_broadcast([P, heads, head_dim])
                      nc.vector.tensor_tensor(out=ndv, in0=uv, in1=rcb,
                                              op=OP.mult)
                      nd2 = ep.tile([P, embed], F32, tag="nodes2")
                      nc.vector.tensor_tensor(out=nd2[:], in0=nd[:], in1=Wb[:],
                                              op=OP.add)
                      nc.sync.dma_start(out=t_out[pos * P:(pos + 1) * P, :],
                                        in_=nd2[:])
    nc.finalize()
    return nc


# ---------------------------------------------------------------- entry

def _run(node_features, edge_features, W_kernel, W_bias, We_kernel, We_bias,
         a, senders, receivers, trace=False):
    n_nodes, in_dim = node_features.shape
    heads, head_dim = a.shape
    embed = heads * head_dim
    edge_dim = edge_features.shape[1]
    plan = _plan(receivers, senders, n_nodes, NCORES)
    n_pad = plan["nb_pad"] * P
    in_maps = _host_inputs(plan, node_features, edge_features, W_kernel,
                           W_bias, We_kernel, We_bias, a, senders)
    nc = _build(plan, n_pad, in_dim, edge_dim, embed, heads)
    res = run_bass_kernel_spmd(nc, in_maps, core_ids=list(range(NCORES)),
                               trace=trace)
    # reassemble: core outputs are [npos*P, embed]; position rows -> blocks
    out = np.zeros((n_pad, embed), np.float32)
    for core in range(NCORES):
        o = res.results[core]["out"]
        for pos in range(plan["npos"]):
            b = int(plan["blocks"][pos, core])
            out[b * P:(b + 1) * P] = o[pos * P:(pos + 1) * P]
    out = out[:n_nodes]
    # nodes with no incoming edges: reference segment_sum gives exactly 0
    deg = np.bincount(receivers.astype(np.int64), minlength=n_nodes)
    if (deg == 0).any():
        out[deg == 0] = 0.0
    return out, res


def kernel(node_features, edge_features, W_kernel, W_bias, We_kernel,
           We_bias, a, senders, receivers):
    node_features = np.asarray(node_features, np.float32)
    edge_features = np.asarray(edge_features, np.float32)
    W_kernel = np.asarray(W_kernel, np.float32)
    W_bias = np.asarray(W_bias, np.float32)
    We_kernel = np.asarray(We_kernel, np.float32)
    We_bias = np.asarray(We_bias, np.float32)
    a = np.asarray(a, np.float32)
    senders = np.asarray(senders, np.int32)
    receivers = np.asarray(receivers, np.int32)
    out, _ = _run(node_features, edge_features, W_kernel, W_bias, We_kernel,
                  We_bias, a, senders, receivers)
    return out



# revision 2
# speedup vs baseline: 2.5903x; 2.5903x over previous
"""GATv2 message-passing kernel for 8 Trainium2 NeuronCores (Bass/Tile), v2.

Strategy (edge parallelism over receiver-sorted edges), changes vs v1:
  * bf16 on every matmul/gather/streaming path (PE runs 1 cycle/row instead
    of 4; streaming DMA halves).  fp32 kept for PSUM accumulation and the
    logits; the mish chain runs bf16 on the vector engine where possible.
  * The receiver one-hot matrices (both orientations) are precomputed on the
    host in fp8e4 and streamed from HBM with ONE DMA per position (GTh:
    [128 rloc, edge] for the h_recv scatter; Gtf: [edge, rloc] per group for
    the segment-sum matmuls).  This removes the ones-broadcast matmul and
    all DVE is_equal builds, and keeps the HWDGE queue short.
  * es transposes into the attention PSUM run as regular matmuls against a
    bf16 identity (is_transpose would force a bf16 PSUM dtype).
  * Phase A (h = nf @ W) is bf16 with a single [128, 1024] PSUM tile per
    sweep, one interleaved nfT DMA, one ACT Copy, one DMA out.
  * CHUNK_G=8 (1024-edge chunks) halves per-instruction overheads.
"""

import ml_dtypes
import numpy as np

import concourse.bass as bass
import concourse.bacc as bacc
import concourse.tile as tile
from concourse import mybir
from concourse.bass_utils import run_bass_kernel_spmd
from concourse.tile_rust import add_dep_helper

F32 = mybir.dt.float32
BF16 = mybir.dt.bfloat16
FP8 = mybir.dt.float8e4
I32 = mybir.dt.int32
AF = mybir.ActivationFunctionType
OP = mybir.AluOpType
BF = ml_dtypes.bfloat16
F8 = ml_dtypes.float8_e4m3fn

# The act-table-load pass assigns each activation the FIRST table set whose
# function list contains it.  By default Exp->exp_and_others and
# Ln->natural_log, which puts this kernel's exp/ln chain in different sets
# and forces a 1.28us table reload between almost every ACT op.  Emptying
# those two sets (ids preserved) makes Exp and Ln co-resolve to
# natural_log_exp_and_others and Tanh to sigmoid_and_others: 2 reloads per
# receiver block instead of ~6.
import concourse.hw_specs as _hw_specs
import concourse.bacc as _bacc_mod

if not hasattr(_hw_specs, "_gat_orig"):
    _hw_specs._gat_orig = _hw_specs.get_activation_tables

    def _gat_patched(arch):
        t = dict(_hw_specs._gat_orig(arch))
        for k in ("exp_and_others", "natural_log"):
            if k in t:
                t[k] = set()
        return t

    _hw_specs.get_activation_tables = _gat_patched
    _bacc_mod.get_activation_tables = _gat_patched

N_NODES = 50000
N_EDGES = 800000
IN_DIM = 256
EDGE_DIM = 64
EMBED = 128
HEADS = 8
HEAD_DIM = EMBED // HEADS
P = 128
NCORES = 8
CHUNK_G = 8  # groups (of 128 edges) per processing chunk
PAD_RLOC = 200.0  # sentinel receiver-local id for padding edges (> 127)


# ---------------------------------------------------------------- host plan

def _plan(receivers, senders, n_nodes, ncores):
    """Sort edges by receiver, then by sender within each 128-node receiver
    block (so gathers use monotone addresses and fit int16 index windows);
    deal blocks to cores balanced by edge count; pad every (core, position)
    to a common group count; pick per-(position, chunk) gather base offsets
    shared by all cores."""
    order = np.argsort(receivers, kind="stable").astype(np.int64)
    r_s = receivers[order].astype(np.int64)
    nb = -(-n_nodes // P)
    npos = -(-nb // ncores)
    nb_pad = npos * ncores
    n_pad = nb_pad * P
    cnt = np.bincount(r_s // P, minlength=nb_pad).astype(np.int64)
    estart = np.zeros(nb_pad, np.int64)
    estart[1:] = np.cumsum(cnt)[:-1]
    # sender-sort within each receiver block
    for b in range(nb_pad):
        e0, c = int(estart[b]), int(cnt[b])
        if c > 1:
            seg = order[e0:e0 + c]
            order[e0:e0 + c] = seg[np.argsort(senders[seg], kind="stable")]
    r_s = receivers[order].astype(np.int64)
    gcnt = np.maximum(-(-cnt // P), 1)
    deal = np.argsort(-gcnt, kind="stable")
    blocks = deal.reshape(npos, ncores)  # blocks[pos, core] -> block id
    gpos = gcnt[blocks].max(axis=1)      # groups per position (same all cores)
    goff = np.zeros(npos, np.int64)
    goff[1:] = np.cumsum(gpos)[:-1]
    # per-(position, chunk) gather windows, uniform across cores.  A gather
    # descriptor is (group offset within position, n groups, base row); when
    # a full chunk's sender window overflows int16, split it in halves.
    def _window(pos, glo, ghi):
        lo, hi = np.iinfo(np.int64).max, 0
        for core in range(ncores):
            b = int(blocks[pos, core])
            e0, c = int(estart[b]), int(cnt[b])
            sc = senders[order[e0:e0 + c]].astype(np.int64)
            part = sc[glo * P:min(ghi * P, c)]
            if part.size:
                lo = min(lo, int(part.min()))
                hi = max(hi, int(part.max()))
        if lo > hi:
            return 0, 0
        return lo, hi

    gdesc = []
    for pos in range(npos):
        gp = int(gpos[pos])
        descs = []
        for ch in range(-(-gp // CHUNK_G)):
            g_lo = ch * CHUNK_G
            g_hi = min(g_lo + CHUNK_G, gp)
            lo, hi = _window(pos, g_lo, g_hi)
            if hi - lo < 32768:
                descs.append((g_lo, g_hi - g_lo, lo))
            else:
                mid = g_lo + (g_hi - g_lo + 1) // 2
                for a_, b_ in ((g_lo, mid), (mid, g_hi)):
                    lo, hi = _window(pos, a_, b_)
                    assert hi - lo < 32768, \
                        f"gather window overflow at pos {pos}: {hi - lo}"
                    descs.append((a_, b_ - a_, lo))
        gdesc.append(descs)
    return dict(order=order, r_s=r_s, cnt=cnt, estart=estart, blocks=blocks,
                gpos=gpos, goff=goff, gtot=int(gpos.sum()),
                ecap=int(gpos.sum()) * P, npos=npos, nb_pad=nb_pad,
                gdesc=gdesc, n_pad=n_pad)


def _host_inputs(plan, node_features, edge_features, W_kernel, W_bias,
                 We_kernel, We_bias, a, senders):
    """Build the per-core input maps (all numpy, no math beyond transposes)."""
    npos, gtot, ecap = plan["npos"], plan["gtot"], plan["ecap"]
    n_pad = plan["nb_pad"] * P
    n_nodes, in_dim = node_features.shape
    heads, head_dim = a.shape
    embed = heads * head_dim
    edge_dim = edge_features.shape[1]
    HW_ = 8 * P  # phase A nodes per sweep

    # interleaved nfT: per sweep of 1024 nodes, k-rows 0:128 then 128:256
    nfT = np.zeros((in_dim, n_pad), np.float32)
    nfT[:, :n_nodes] = node_features.T
    nfT2 = np.zeros((P, 2 * n_pad), BF)
    for s in range(n_pad // HW_):
        nfT2[:, 2 * s * HW_:2 * s * HW_ + HW_] = nfT[0:P, s * HW_:(s + 1) * HW_]
        nfT2[:, 2 * s * HW_ + HW_:2 * (s + 1) * HW_] = \
            nfT[P:2 * P, s * HW_:(s + 1) * HW_]
    We_aug = np.concatenate(
        [We_kernel, (We_bias + 2.0 * W_bias)[None, :]], axis=0
    ).astype(BF)
    A_blk = np.zeros((embed, heads), np.float32)
    for h in range(heads):
        A_blk[h * head_dim:(h + 1) * head_dim, h] = a[h]
    Wb_rep = np.tile(W_bias[None, :], (P, 1)).astype(np.float32)
    identity = np.eye(P, dtype=BF)

    efT_all = np.ascontiguousarray(edge_features[plan["order"]].T).astype(BF)
    s_sorted = senders[plan["order"]].astype(np.int32)
    rloc_all = (plan["r_s"] - (plan["r_s"] // P) * P).astype(np.int64)

    qrange = np.arange(P, dtype=np.int64)
    shared = {
        "nfT2": nfT2, "W": W_kernel.astype(BF), "We_aug": We_aug,
        "A_blk": A_blk.astype(BF), "Wb_rep": Wb_rep, "identity": identity,
    }
    in_maps = []
    for core in range(NCORES):
        senders16 = np.zeros((P, gtot * 8), np.int16)
        efTa = np.zeros((edge_dim + 1, ecap), BF)
        efTa[edge_dim, :] = 1.0
        rl_flat = np.full(ecap, int(PAD_RLOC), np.int64)
        blocknodes = np.zeros((P, npos), np.int32)
        for pos in range(npos):
            b = int(plan["blocks"][pos, core])
            g0 = int(plan["goff"][pos])
            gp = int(plan["gpos"][pos])
            c = int(plan["cnt"][b])
            e0 = int(plan["estart"][b])
            blocknodes[:, pos] = b * P + np.arange(P)
            col0 = g0 * P
            efTa[:edge_dim, col0:col0 + c] = efT_all[:, e0:e0 + c]
            rl_flat[col0:col0 + c] = rloc_all[e0:e0 + c]
            for (g_lo, ng, base) in plan["gdesc"][pos]:
                s_ch = ng * P
                tmp_s = np.full(s_ch, base, np.int64)  # pads -> row `base`
                r0 = g_lo * P
                nreal = min(max(c - r0, 0), s_ch)
                tmp_s[:nreal] = s_sorted[e0 + r0:e0 + r0 + nreal]
                rel = (tmp_s - base).astype(np.int16)
                blk16 = np.tile(rel.reshape(s_ch // 16, 16).T, (8, 1))
                cb = (g0 * P + r0) // 16
                senders16[:, cb:cb + s_ch // 16] = blk16
        # receiver one-hots, both orientations, fp8 (pads -> all-zero),
        # packed per position as [GTh_pos | Gtf_pos] for one DMA each
        GTh = (rl_flat[None, :] == qrange[:, None])
        rl_g = rl_flat.reshape(gtot, P)  # [g, p] -> rloc of edge g*128+p
        Gtf = (rl_g[:, :, None] == qrange[None, None, :])  # [g, p, q]
        GG = np.zeros((P, 2 * ecap), F8)
        for pos in range(npos):
            g0 = int(plan["goff"][pos])
            gp = int(plan["gpos"][pos])
            c0 = 2 * g0 * P
            GG[:, c0:c0 + gp * P] = GTh[:, g0 * P:(g0 + gp) * P].astype(F8)
            GG[:, c0 + gp * P:c0 + 2 * gp * P] = np.ascontiguousarray(
                Gtf[g0:g0 + gp].transpose(1, 0, 2).reshape(P, gp * P)
            ).astype(F8)
        m = dict(shared)
        m.update({"senders16": senders16, "efTa": efTa,
                  "GG": GG, "blocknodes": blocknodes})
        in_maps.append(m)
    return in_maps


# ---------------------------------------------------------------- bass build

def _build(plan, n_pad, in_dim, edge_dim, embed, heads, debug=False,
           repeat=1, parts="full"):
    head_dim = embed // heads
    npos, gtot, ecap = plan["npos"], plan["gtot"], plan["ecap"]
    gpos, goff = plan["gpos"], plan["goff"]
    gpmax = int(gpos.max())
    UW = embed + heads  # U columns: [weighted sum | denom]

    nc = bacc.Bacc("TRN2", num_swdge_queues=4,
                   dynamic_dma_scratch_size=65536)
    t_nfT2 = nc.dram_tensor("nfT2", [P, 2 * n_pad], BF16,
                            kind="ExternalInput")
    t_W = nc.dram_tensor("W", [in_dim, embed], BF16, kind="ExternalInput")
    t_We = nc.dram_tensor("We_aug", [edge_dim + 1, embed], BF16,
                          kind="ExternalInput")
    t_A = nc.dram_tensor("A_blk", [embed, heads], BF16, kind="ExternalInput")
    t_Wb = nc.dram_tensor("Wb_rep", [P, embed], F32, kind="ExternalInput")
    t_id = nc.dram_tensor("identity", [P, P], BF16, kind="ExternalInput")
    t_s16 = nc.dram_tensor("senders16", [P, gtot * 8], mybir.dt.int16,
                           kind="ExternalInput")
    t_efT = nc.dram_tensor("efTa", [edge_dim + 1, ecap], BF16,
                           kind="ExternalInput")
    t_GG = nc.dram_tensor("GG", [P, 2 * ecap], FP8, kind="ExternalInput")
    t_bn = nc.dram_tensor("blocknodes", [P, npos], I32, kind="ExternalInput")
    t_out = nc.dram_tensor("out", [npos * P, embed], F32,
                           kind="ExternalOutput")
    t_h = nc.dram_tensor("h_scratch", [n_pad, embed], BF16, kind="Internal")

    with tile.TileContext(nc) as tc:
        with tc.tile_pool(name="const", bufs=1) as cp:
            def cload(t, shape):
                s = cp.tile(shape, t.dtype, tag=f"c_{t.name}")
                nc.sync.dma_start(out=s[:], in_=t[:])
                return s

            W0 = cp.tile([P, embed], BF16)
            nc.sync.dma_start(out=W0[:], in_=t_W[0:P, :])
            W1 = cp.tile([P, embed], BF16)
            nc.sync.dma_start(out=W1[:], in_=t_W[P:2 * P, :])
            We = cload(t_We, [edge_dim + 1, embed])
            Ab = cload(t_A, [embed, heads])
            Wb = cload(t_Wb, [P, embed])
            idn = cload(t_id, [P, P])
            s16 = cload(t_s16, [P, gtot * 8])
            bn = cload(t_bn, [P, npos])

            # ---------------- phase A: h = nf @ W (no bias) ----------------
            for _rep in range(repeat):
              with tc.tile_pool(name=f"ha{_rep}", bufs=4) as hap, \
                      tc.tile_pool(name=f"haps{_rep}", bufs=3, space="PSUM") as hpp:
                  HW_ = 8 * P  # nodes per sweep
                  for nt in range(n_pad // HW_):
                      na = hap.tile([P, 2 * HW_], BF16, tag="nfT0")
                      nc.sync.dma_start(
                          out=na[:],
                          in_=t_nfT2[:, 2 * nt * HW_:2 * (nt + 1) * HW_])
                      hstage = hap.tile([P, HW_], BF16, tag="hstage")
                      hp = hpp.tile([P, HW_], F32, tag="hps")
                      for t in range(HW_ // P):
                          nc.tensor.matmul(hp[:, t * P:(t + 1) * P],
                                           lhsT=na[:, t * P:(t + 1) * P],
                                           rhs=W0[:], start=True, stop=False)
                          nc.tensor.matmul(hp[:, t * P:(t + 1) * P],
                                           lhsT=na[:, HW_ + t * P:
                                                    HW_ + (t + 1) * P],
                                           rhs=W1[:], start=False, stop=True)
                      nc.scalar.activation(out=hstage[:], in_=hp[:],
                                           func=AF.Copy)
                      out_view = bass.AP(
                          t_h[:].tensor, nt * HW_ * embed,
                          [[embed, P], [P * embed, HW_ // P], [1, embed]])
                      nc.vector.dma_start(out=out_view, in_=hstage[:])

              tc.strict_bb_all_engine_barrier()
              if parts == "a":
                  continue

              # ---------------- phase B: edge processing ---------------------
              with tc.tile_pool(name=f"eb{_rep}", bufs=6) as ep, \
                      tc.tile_pool(name=f"ebsm{_rep}", bufs=3) as esm, \
                      tc.tile_pool(name=f"ebp{_rep}", bufs=2, space="PSUM") as pp, \
                      tc.tile_pool(name=f"ups{_rep}", bufs=2, space="PSUM") as up:
                  prev_exp_ins = None
                  for pos in range(npos):
                      g_here = int(gpos[pos])
                      g0 = int(goff[pos])
                      Hb = ep.tile([P, embed], BF16, tag="Hb")
                      nc.gpsimd.indirect_dma_start(
                          out=Hb[:], out_offset=None, in_=t_h[:],
                          in_offset=bass.IndirectOffsetOnAxis(
                              ap=bn[:, pos:pos + 1], axis=0))
                      # one DMA per position for both one-hots and ef
                      gg = ep.tile([P, 2 * gpmax * P], FP8, tag="gg",
                                   bufs=2)
                      nc.sync.dma_start(
                          out=gg[:, :2 * g_here * P],
                          in_=t_GG[:, 2 * g0 * P:2 * (g0 + g_here) * P])
                      efp = ep.tile([edge_dim + 1, gpmax * P], BF16,
                                    tag="efp", bufs=2)
                      nc.sync.dma_start(
                          out=efp[:, :g_here * P],
                          in_=t_efT[:, g0 * P:(g0 + g_here) * P])
                      Ups = up.tile([P, UW], F32, tag="U", bufs=3)
                      lgb = up.tile([P, ((gpmax * heads + 127) // 128)
                                     * 128], F32, tag="lgb", bufs=1)
                      nchunks = -(-g_here // CHUNK_G)
                      es_tiles = []
                      sp_tiles = []
                      xc_tiles = []
                      # --- stage 1: pre-activation x and softplus(x) ---------
                      # (ACT stays on the {exp, ln} table set here)
                      for c in range(nchunks):
                          gc = min(CHUNK_G, g_here - c * CHUNK_G)
                          s = gc * P
                          co = c * CHUNK_G * P       # column offset in block
                          es = ep.tile([P, CHUNK_G * P], BF16, tag="es",
                                       bufs=6)
                          es_tiles.append(es)
                          for (g_lo, ng, base) in plan["gdesc"][pos]:
                              if not (c * CHUNK_G <= g_lo < c * CHUNK_G + gc):
                                  continue
                              rows = min(n_pad - base, 32768)
                              cb = g0 * 8 + g_lo * 8
                              o0 = (g_lo - c * CHUNK_G) * P
                              sg = ng * P
                              nc.gpsimd.dma_gather(
                                  out_ap=es[:, o0:o0 + sg].rearrange(
                                      "p (j e) -> p j e", e=embed),
                                  in_ap=t_h[base:base + rows, :],
                                  idxs_ap=s16[:, cb:cb + sg // 16],
                                  num_idxs=sg, num_idxs_reg=sg,
                                  elem_size=embed)
                          if parts == "ag":
                              continue
                          at = pp.tile([P, CHUNK_G * P], F32, tag="attnT",
                                       bufs=2)
                          nc.tensor.matmul(at[:, :s], lhsT=We[:],
                                           rhs=efp[:, co:co + s],
                                           start=True, stop=False)
                          nc.tensor.matmul(at[:, :s], lhsT=Hb[:],
                                           rhs=gg[:, co:co + s], start=False,
                                           stop=False)
                          for j in range(gc):
                              # es_j^T via regular matmul against identity
                              # (is_transpose would force bf16 PSUM out)
                              nc.tensor.matmul(
                                  at[:, j * P:(j + 1) * P],
                                  lhsT=es[:, j * P:(j + 1) * P], rhs=idn[:],
                                  start=False, stop=(j == gc - 1))
                          # mish(x) = x * tanh(ln(1 + exp(x))) — composed from
                          # table-mapped functions.  xc copies x out of PSUM.
                          xc = ep.tile([P, CHUNK_G * P], BF16, tag="xc",
                                       bufs=6)
                          xc_tiles.append(xc)
                          nc.vector.tensor_copy(out=xc[:, :s], in_=at[:, :s])
                          vv = ep.tile([P, CHUNK_G * P], F32, tag="vv", bufs=4)
                          v_ins = nc.scalar.activation(out=vv[:, :s],
                                                       in_=at[:, :s],
                                                       func=AF.Exp)
                          if c == 0 and prev_exp_ins is not None:
                              # keep ACT's {exp,ln} ops grouped after the
                              # previous block's {tanh,exp} ops so lower_act
                              # emits 2 table loads per block, not ~7.
                              add_dep_helper(v_ins.ins, prev_exp_ins,
                                             sync=False,
                                             reason="act table grouping")
                          sp = ep.tile([P, CHUNK_G * P], BF16, tag="sp",
                                       bufs=6)
                          sp_tiles.append(sp)
                          sp_ins = nc.scalar.activation(out=sp[:, :s],
                                                        in_=vv[:, :s],
                                                        func=AF.Ln, bias=1.0)
                      if parts == "ag":
                          continue
                      # --- stage 2: tanh, mish, logits -----------------------
                      # (ACT switches to the {tanh, exp} table set)
                      for c in range(nchunks):
                          gc = min(CHUNK_G, g_here - c * CHUNK_G)
                          s = gc * P
                          mi = ep.tile([P, CHUNK_G * P], BF16, tag="mish", bufs=4)
                          t_ins = nc.scalar.activation(out=mi[:, :s],
                                                       in_=sp_tiles[c][:, :s],
                                                       func=AF.Tanh)
                          if c == 0:
                              add_dep_helper(t_ins.ins, sp_ins.ins, sync=False,
                                             reason="act table grouping")
                          nc.vector.tensor_tensor(out=mi[:, :s],
                                                  in0=xc_tiles[c][:, :s],
                                                  in1=mi[:, :s], op=OP.mult)
                          for j in range(gc):
                              nc.tensor.matmul(
                                  lgb[:, (c * CHUNK_G + j) * heads:
                                      (c * CHUNK_G + j + 1) * heads],
                                  lhsT=mi[:, j * P:(j + 1) * P], rhs=Ab[:],
                                  start=True, stop=True)
                      exb = esm.tile([P, gpmax * heads], BF16, tag="exb")
                      exp_ins = nc.scalar.activation(
                          out=exb[:, :g_here * heads],
                          in_=lgb[:, :g_here * heads], func=AF.Exp)
                      prev_exp_ins = exp_ins.ins
                      # --- stage 3: weighted scatter-accumulate ---
                      for c in range(nchunks):
                          gc = min(CHUNK_G, g_here - c * CHUNK_G)
                          s = gc * P
                          co = c * CHUNK_G * P
                          es = es_tiles[c]
                          rb = ep.tile([P, CHUNK_G * UW], BF16, tag="rhsb", bufs=4)
                          rb3 = rb[:].rearrange("p (j c) -> p j c", j=CHUNK_G)
                          ex_view = rb3[:, :gc, embed:UW]
                          exb_view = exb[:, c * CHUNK_G * heads:
                                         (c * CHUNK_G + gc) * heads].rearrange(
                              "p (j h) -> p j h", j=gc)
                          nc.vector.tensor_copy(out=ex_view, in_=exb_view)
                          m_view = rb3[:, :gc, 0:embed].rearrange(
                              "p j (h w) -> p j h w", w=head_dim)
                          es_view = es[:, :s].rearrange(
                              "p (j h w) -> p j h w", j=gc, w=head_dim)
                          ex_b = exb_view.to_broadcast([P, gc, heads, head_dim])
                          nc.vector.tensor_tensor(out=m_view, in0=es_view,
                                                  in1=ex_b, op=OP.mult)
                          for j in range(gc):
                              nc.tensor.matmul(
                                  Ups[:],
                                  lhsT=gg[:, (g_here + c * CHUNK_G + j) * P:
                                          (g_here + c * CHUNK_G + j + 1) * P],
                                  rhs=rb[:, j * UW:(j + 1) * UW],
                                  start=(c == 0 and j == 0),
                                  stop=(c == nchunks - 1 and j == gc - 1))
                      # ---- block epilogue: out = U / max(denom, eps) + Wb ----
                      dn = ep.tile([P, heads], F32, tag="dn")
                      nc.vector.tensor_scalar(out=dn[:],
                                              in0=Ups[:, embed:UW],
                                              scalar1=1e-30, scalar2=None,
                                              op0=OP.max)
                      rc = ep.tile([P, heads], F32, tag="rc")
                      nc.vector.reciprocal(rc[:], dn[:])
                      nd = ep.tile([P, embed], F32, tag="nodes")
                      ndv = nd[:].rearrange("p (h w) -> p h w", w=head_dim)
                      uv = Ups[:, 0:embed].rearrange("p (h w) -> p h w",
                                                     w=head_dim)
                      rcb = rc[:].to_broadcast([P, heads, head_dim])
                      nc.vector.tensor_tensor(out=ndv, in0=uv, in1=rcb,
                                              op=OP.mult)
                      nd2 = ep.tile([P, embed], F32, tag="nodes2")
                      nc.vector.tensor_tensor(out=nd2[:], in0=nd[:], in1=Wb[:],
                                              op=OP.add)
                      nc.sync.dma_start(out=t_out[pos * P:(pos + 1) * P, :],
                                        in_=nd2[:])
    nc.finalize()
    return nc


# ---------------------------------------------------------------- entry

def _run(node_features, edge_features, W_kernel, W_bias, We_kernel, We_bias,
         a, senders, receivers, trace=False):
    n_nodes, in_dim = node_features.shape
    heads, head_dim = a.shape
    embed = heads * head_dim
    edge_dim = edge_features.shape[1]
    plan = _plan(receivers, senders, n_nodes, NCORES)
    n_pad = plan["nb_pad"] * P
    in_maps = _host_inputs(plan, node_features, edge_features, W_kernel,
                           W_bias, We_kernel, We_bias, a, senders)
    nc = _build(plan, n_pad, in_dim, edge_dim, embed, heads)
    res = run_bass_kernel_spmd(nc, in_maps, core_ids=list(range(NCORES)),
                               trace=trace)
    # reassemble: core outputs are [npos*P, embed]; position rows -> blocks
    out = np.zeros((n_pad, embed), np.float32)
    for core in range(NCORES):
        o = res.results[core]["out"]
        for pos in range(plan["npos"]):
            b = int(plan["blocks"][pos, core])
            out[b * P:(b + 1) * P] = o[pos * P:(pos + 1) * P]
    out = out[:n_nodes]
    # nodes with no incoming edges: reference segment_sum gives exactly 0
    deg = np.bincount(receivers.astype(np.int64), minlength=n_nodes)
    if (deg == 0).any():
        out[deg == 0] = 0.0
    return out, res


def kernel(node_features, edge_features, W_kernel, W_bias, We_kernel,
           We_bias, a, senders, receivers):
    node_features = np.asarray(node_features, np.float32)
    edge_features = np.asarray(edge_features, np.float32)
    W_kernel = np.asarray(W_kernel, np.float32)
    W_bias = np.asarray(W_bias, np.float32)
    We_kernel = np.asarray(We_kernel, np.float32)
    We_bias = np.asarray(We_bias, np.float32)
    a = np.asarray(a, np.float32)
    senders = np.asarray(senders, np.int32)
    receivers = np.asarray(receivers, np.int32)
    out, _ = _run(node_features, edge_features, W_kernel, W_bias, We_kernel,
                  We_bias, a, senders, receivers)
    return out
